# revision 12
# baseline (speedup 1.0000x reference)
"""Trainium2 Bass kernel for the SelfOrg spiking-network step.

Reference computation (per batch b, neuron n):
    z_out_new = BETA * z_out + z
    z_loo[b,j,n] = z_out_new[b, j + (j>=n)]            (leave-one-out gather)
    drive[b,n]  = sum_k x[b,k,n] * w[k,n]  (k < N_IN)
                + sum_j z_loo[b,j,n] * w[N_IN+j, n]
    v_new = ALPHA*v + drive - V_TH*z
    z_new = (v_new - V_TH > 0)

Strategy:
  * Batch-parallel over 8 cores (8 batches each). The kernel is memory
    bound on streaming x; x and w are cast to fp16 on the host, which
    halves HBM traffic (error ~2^-11 per term, far inside the 2e-2
    tolerance) and unlocks the DVE 2x packed mode and the PE 16-bit
    1-column/cycle rate.
  * The x-part is an elementwise-weighted reduction over k. Layout: k on
    SBUF partitions (p = k//16, s = k%16), n in the free dim. The vector
    engine squares x against w in place; the tensor engine reduces over
    partitions with a per-batch indicator stationary operand
    (lhsT[p, m] = (m==b)), accumulating into one (8,512) PSUM tile.
  * The leave-one-out term is algebraically a dense matmul
    z_out_new @ Wf where Wf[m,n] = w[N_IN + m - (m>n), n], diag(Wf)=0.
    Wf is precomputed on the host; its 4 matmuls close the same PSUM
    accumulation group at the tail.
  * The HW bottleneck at this size is the sync sequencer (descriptor
    issue ~0.6us each + semaphore ops), not DMA bandwidth: one HWDGE
    ring already saturates the 16 SDMA engines. So: few instructions,
    one ring, variable chunking - batch 0 in 8 small chunks (fast ramp),
    batches 1-6 as single 2MB DMAs, batch 7 in 4 chunks (short drain).
"""

import numpy as np

# model hyperparameters (must match the reference)
N_IN = 2048
NN = 512
BATCH = 64
DT, TAU_M, TAU_X = 0.05, 10.0, 2.0
ALPHA = 1.0 - DT / TAU_M
BETA = 1.0 - DT / TAU_X
V_TH = 2.0

NCORES = 8
BPC = BATCH // NCORES      # batches per core
P = 128                    # SBUF partitions
S = N_IN // P              # 16 k-rows folded per partition
FD = S * NN                # 8192 free elements of one batch tile
# per-batch chunk counts: fast ramp, fat middle, short drain
BCHUNKS = [8, 1, 1, 1, 1, 1, 1, 4]


def _build_nc():
    import concourse.mybir as mybir
    from concourse import bacc
    from concourse.masks import make_identity
    from concourse.tile import TileContext

    f32 = mybir.dt.float32
    f16 = mybir.dt.float16
    nc = bacc.Bacc("TRN2", name="selforg_step")

    x_h = nc.dram_tensor("x", [BPC, N_IN, NN], f16, kind="ExternalInput")
    st_h = nc.dram_tensor("state", [3, BPC, NN], f32, kind="ExternalInput")
    w_h = nc.dram_tensor("w", [N_IN, NN], f16, kind="ExternalInput")
    wf_h = nc.dram_tensor("wf", [NN, NN], f16, kind="ExternalInput")
    ind_h = nc.dram_tensor("ind", [P, BPC * BPC], f16, kind="ExternalInput")
    out_h = nc.dram_tensor("out", [BPC, 3, NN], f32, kind="ExternalOutput")

    # partition p <- x[b] bytes [16KB*p, 16KB*(p+1)): k = 16p + s
    x_r = x_h[:, :, :].rearrange("b (p s) n -> b p (s n)", p=P)
    w_r = w_h[:, :].rearrange("(p s) n -> p (s n)", p=P)
    wf_r = wf_h[:, :].rearrange("(t p) n -> p t n", p=P)
    st_r = st_h[:, :, :].rearrange("t b n -> b t n")
    out_r = out_h[:, :, :].rearrange("b t n -> b (t n)")

    with TileContext(nc) as tc:
        with (
            tc.tile_pool(name="const", bufs=1) as cpool,
            tc.tile_pool(name="xs", bufs=4) as xpool_s,     # 256KB ramp chunks
            tc.tile_pool(name="xb", bufs=3) as xpool_b,     # 2MB bulk chunks
            tc.tile_pool(name="xd", bufs=4) as xpool_d,     # 512KB drain chunks
            tc.tile_pool(name="psum", bufs=1, space="PSUM") as ppool,
            tc.tile_pool(name="psum2", bufs=2, space="PSUM") as ppool2,
        ):
            # ---- input DMAs, all on the sync HWDGE ring in need-order.
            # (One ring already saturates the 16 SDMA engines; a second
            # ring only fair-shares them and starves the critical head.)
            st_sb = cpool.tile([BPC, 3 * NN], f32)
            zo_in = st_sb[:, 0:NN]
            z_in = st_sb[:, NN : 2 * NN]
            v_in = st_sb[:, 2 * NN : 3 * NN]
            nc.sync.dma_start(st_sb[:, :].rearrange("b (t n) -> b t n", t=3), st_r)
            ind = cpool.tile([P, BPC * BPC], f16)
            nc.sync.dma_start(ind[:, :], ind_h[:, :])
            w_sb = cpool.tile([P, FD], f16)
            wf_sb = cpool.tile([P, 4 * NN], f16)

            ident = cpool.tile([BPC, BPC], f32)
            make_identity(nc, ident[:, :])

            # ---- output staging tile: [vn | zn | zon] in the free dim
            res = cpool.tile([BPC, 3 * NN], f32)
            vn = res[:, 0:NN]
            zn = res[:, NN : 2 * NN]
            zon = res[:, 2 * NN : 3 * NN]

            # lateral trace update (needed by the PE transposes)
            nc.vector.tensor_scalar_mul(zon, zo_in, BETA)
            nc.vector.tensor_add(zon, zon, z_in)

            zonT = cpool.tile([P, 4 * BPC], f16)
            av_sb = cpool.tile([BPC, NN], f32)
            zv_sb = cpool.tile([BPC, NN], f32)

            # ---- main loop: drive[b,n] = sum_k x[b,k,n]*w[k,n] ----
            # One PSUM accumulation group: all indicator matmuls, then
            # the 4 lateral matmuls close it at the tail.
            psum_drive = ppool.tile([BPC, NN], f32, tag="drive")
            total_mms = BPC * S + 4
            mm_idx = 0
            pools = {8: xpool_s, 4: xpool_d, 1: xpool_b}
            for b in range(BPC):
                chunks = BCHUNKS[b]
                cfd = FD // chunks
                for c in range(chunks):
                    cs = slice(c * cfd, (c + 1) * cfd)
                    if b == 0:
                        # stream w just ahead of the x chunk using it
                        nc.sync.dma_start(w_sb[:, cs], w_r[:, cs])
                    xc = pools[chunks].tile([P, cfd], f16, tag="xc")
                    nc.sync.dma_start(xc[:, :], x_r[b, :, cs])
                    if b == 0 and c == chunks - 1:
                        # wf rides mid-stream; only the tail mms need it
                        nc.sync.dma_start(
                            wf_sb[:, :].rearrange("p (t n) -> p t n", t=4),
                            wf_r[:, :, :],
                        )
                    # in-place product; the PE consumes xc directly
                    nc.vector.tensor_mul(xc[:, :], xc[:, :], w_sb[:, cs])
                    for j in range(cfd // NN):
                        nc.tensor.matmul(
                            psum_drive[:, :],
                            ind[:, BPC * b : BPC * (b + 1)],
                            xc[:, j * NN : (j + 1) * NN],
                            start=(mm_idx == 0),
                            stop=False,
                        )
                        mm_idx += 1
                if b == 0:
                    # mid-stream slack: PE transposes of zon (4x (8,128)
                    # -> (128,8), cast fp16) + av = ALPHA*v - V_TH*z
                    for t in range(4):
                        psum_t = ppool2.tile([P, BPC], f32, tag="tr")
                        nc.tensor.transpose(
                            psum_t[:, :], zon[:, t * P : (t + 1) * P], ident[:, :]
                        )
                        nc.vector.tensor_copy(
                            zonT[:, t * BPC : (t + 1) * BPC], psum_t[:, :]
                        )
                    nc.vector.tensor_scalar_mul(av_sb[:, :], z_in, -V_TH)
                    nc.vector.tensor_scalar_mul(zv_sb[:, :], v_in, ALPHA)
                    nc.vector.tensor_add(av_sb[:, :], av_sb[:, :], zv_sb[:, :])

            # lateral drive closes the accumulation group
            for t in range(4):
                nc.tensor.matmul(
                    psum_drive[:, :],
                    zonT[:, t * BPC : (t + 1) * BPC],
                    wf_sb[:, t * NN : (t + 1) * NN],
                    start=False,
                    stop=(t == 3),
                )

            # ---- epilogue ----
            nc.vector.tensor_add(vn, av_sb[:, :], psum_drive[:, :])
            nc.vector.tensor_scalar(
                out=zn,
                in0=vn,
                scalar1=V_TH,
                scalar2=None,
                op0=mybir.AluOpType.is_gt,
            )
            nc.scalar.dma_start(out_r, res[:, :])

    return nc


def _make_wf(w: np.ndarray) -> np.ndarray:
    """Wf[m,n] = w[N_IN + m - (m>n), n] off-diagonal, 0 on the diagonal."""
    wl = w[N_IN:]
    m = np.arange(NN)[:, None]
    n = np.arange(NN)[None, :]
    idx = np.minimum(np.where(m > n, m - 1, m), NN - 2)
    return np.where(m == n, np.float32(0.0), wl[idx, n]).astype(np.float32)


def _make_ind() -> np.ndarray:
    """Indicator columns: ind[:, BPC*b + j] = (j == b)."""
    ind = np.zeros((P, BPC * BPC), dtype=np.float16)
    for b in range(BPC):
        ind[:, BPC * b + b] = 1.0
    return ind


def _make_in_maps(x, v, z, z_out, w):
    w16 = np.ascontiguousarray(w[:N_IN]).astype(np.float16)
    wf16 = _make_wf(np.asarray(w, dtype=np.float32)).astype(np.float16)
    ind = _make_ind()
    x16 = np.asarray(x).astype(np.float16)
    state = np.stack(
        [
            np.asarray(z_out, dtype=np.float32),
            np.asarray(z, dtype=np.float32),
            np.asarray(v, dtype=np.float32),
        ]
    )
    in_maps = []
    for c in range(NCORES):
        sl = slice(c * BPC, (c + 1) * BPC)
        in_maps.append(
            {
                "x": np.ascontiguousarray(x16[sl]),
                "state": np.ascontiguousarray(state[:, sl]),
                "w": w16,
                "wf": wf16,
                "ind": ind,
            }
        )
    return in_maps


def run(x, v, z, z_out, w, trace=False):
    """Build + run on the 8 NeuronCores; returns (output, BassKernelResults)."""
    from concourse.bass_utils import run_bass_kernel_spmd

    nc = _build_nc()
    if not nc.is_finalized():
        nc.finalize()
    in_maps = _make_in_maps(x, v, z, z_out, w)
    res = run_bass_kernel_spmd(nc, in_maps, core_ids=list(range(NCORES)), trace=trace)
    # per-core out is [BPC, 3, NN]; reassemble to [3, BATCH, NN]
    full = np.concatenate([r["out"].transpose(1, 0, 2) for r in res.results], axis=1)
    return np.ascontiguousarray(full, dtype=np.float32), res


def kernel(x, v, z, z_out, w):
    out, _ = run(x, v, z, z_out, w)
    return out
